# revision 1
# baseline (speedup 1.0000x reference)
"""Sparse-weight matmul (BiologicalModule) on 8 Trainium2 NeuronCores.

Computes: out = tanh(x @ scatter_coo(kernel_vector, nonzero_ind) + bias)
  x [32, 30000] f32, 500K COO nonzeros into a [30000, 2048] weight matrix.

Strategy (units-sharded, 256 output columns per core):
  - Never materialize the dense [30000, 2048] weight matrix (245 MB). In CSC
    view, out_T[c, :] = sum_k v[c,k] * x[:, r[c,k]].
  - kernel() packs, per core, a padded-CSC entry payload: for each output
    column its entry values and the x column-vectors those entries touch
    (columns mapped to SBUF partitions; entry slots padded to KP, chunked,
    and stored entry-innermost [col, chunk, batch, k]). This is pure data
    layout / sharding prep - no arithmetic.
  - Each core streams its ~4 MB fp16 payload and does all the math on-chip.
    The entry-innermost layout keeps every tensor_tensor operand 2-byte with
    a step-1 inner dim (the value broadcast is a step-0 *middle* dim), which
    enables the DVE 2x_1P perf mode for the multiply. DVE reduces over the
    entry axis (f32 accumulation); ~1/4 of chunks run multiply + add-tree on
    the otherwise-idle GPSIMD engine (f32 product there); ACT applies fused
    bias + tanh. Chunks overlap DMA / DVE / GPSIMD.
"""

import sys

import numpy as np

_TRN_REPO = "/opt/trn_rl_repo"
if _TRN_REPO not in sys.path:
    sys.path.insert(0, _TRN_REPO)

INPUT_DIM = 30000
UNITS = 2048
BATCH = 32
N_CORES = 8
UNITS_PER_CORE = UNITS // N_CORES  # 256
BLOCKS_PER_CORE = UNITS_PER_CORE // 128  # 2
K_CHUNK = 32  # entry-slots per DMA/compute chunk
# Engine per chunk (cycled): D = DVE mul + add-tree + reduce, A = GPSIMD
# mul + add-tree. 3 of 16 chunks on GPSIMD balances its slower tensor_tensor
# against the 2x-mode DVE path.
ENGINE_PATTERN = "DDDDADDDDADDDDAD"
WORK_BUFS = 8

_PROGRAM_CACHE = {}


def _build_program(kp):
    """Build + compile the SPMD bass program for padded column length kp."""
    from concourse import bacc, tile
    from concourse.bass import AP
    import concourse.mybir as mybir

    assert kp % K_CHUNK == 0
    nch = kp // K_CHUNK
    f32 = mybir.dt.float32
    f16 = mybir.dt.float16

    nc = bacc.Bacc("TRN2", target_bir_lowering=False, debug=False,
                   num_devices=N_CORES)
    g_d = nc.dram_tensor("gvals", [BLOCKS_PER_CORE, 128, nch, BATCH, K_CHUNK],
                         f16, kind="ExternalInput")
    vals_d = nc.dram_tensor("vals", [BLOCKS_PER_CORE, 128, kp], f16,
                            kind="ExternalInput")
    bias_d = nc.dram_tensor("bias2", [128, BLOCKS_PER_CORE], f32,
                            kind="ExternalInput")
    out_d = nc.dram_tensor("out", [BLOCKS_PER_CORE, 128, BATCH], f32,
                           kind="ExternalOutput")

    with tile.TileContext(nc) as tc:
        with (
            tc.tile_pool(name="persist", bufs=1) as persist,
            tc.tile_pool(name="work", bufs=WORK_BUFS) as work,
        ):
            bias_t = persist.tile([128, BLOCKS_PER_CORE], f32, tag="bias")
            nc.sync.dma_start(bias_t[:], bias_d[:])
            gidx = 0
            for blk in range(BLOCKS_PER_CORE):
                val_t = persist.tile([128, kp], f16, tag=f"val{blk}",
                                     name=f"val{blk}")
                nc.sync.dma_start(val_t[:], vals_d[blk])
                pt_t = persist.tile([128, nch, BATCH], f32, tag=f"pt{blk}",
                                    name=f"pt{blk}")
                for ch in range(nch):
                    k0 = ch * K_CHUNK
                    g_t = work.tile([128, BATCH, K_CHUNK], f16, tag="g",
                                    name=f"g{blk}_{ch}")
                    nc.sync.dma_start(g_t[:], g_d[blk, :, ch])
                    use_gp = ENGINE_PATTERN[gidx % len(ENGINE_PATTERN)] == "A"
                    gidx += 1
                    # value operand viewed [p, batch(step 0), k(step 1)]
                    base = val_t[:, k0:k0 + K_CHUNK]
                    v_bk = AP(base.tensor, base.offset,
                              [base.ap[0], [0, BATCH], base.ap[1]])
                    if use_gp:
                        prod = work.tile([128, BATCH, K_CHUNK], f32,
                                         tag="prodA", name=f"prodA{blk}_{ch}")
                        nc.gpsimd.tensor_tensor(prod[:], g_t[:], v_bk,
                                                mybir.AluOpType.mult)
                        w = K_CHUNK
                        while w > 1:
                            h = w // 2
                            nc.gpsimd.tensor_tensor(
                                prod[:, :, :h], prod[:, :, :h],
                                prod[:, :, h:w], mybir.AluOpType.add)
                            w = h
                        nc.gpsimd.tensor_copy(pt_t[:, ch, :], prod[:, :, 0])
                    else:
                        prod = work.tile([128, BATCH, K_CHUNK], f16,
                                         tag="prodD", name=f"prodD{blk}_{ch}")
                        nc.vector.tensor_tensor(prod[:], g_t[:], v_bk,
                                                mybir.AluOpType.mult)
                        # fp16 2x-mode add-tree down to 4 lanes, then a f32
                        # tail reduce for the actual accumulation.
                        with nc.allow_low_precision(
                                "fp16 tree partials; f32 tail reduce"):
                            w = K_CHUNK
                            while w > 4:
                                h = w // 2
                                nc.vector.tensor_tensor(
                                    prod[:, :, :h], prod[:, :, :h],
                                    prod[:, :, h:w], mybir.AluOpType.add)
                                w = h
                        nc.vector.tensor_reduce(
                            pt_t[:, ch, :], prod[:, :, :4],
                            mybir.AxisListType.X, mybir.AluOpType.add)
                red = work.tile([128, BATCH], f32, tag="red", name=f"red{blk}")
                nc.vector.tensor_reduce(
                    red[:],
                    pt_t[:].rearrange("p c b -> p b c"),
                    mybir.AxisListType.X,
                    mybir.AluOpType.add,
                )
                outp = work.tile([128, BATCH], f32, tag="outp",
                                 name=f"outp{blk}")
                nc.scalar.activation(
                    outp[:], red[:], mybir.ActivationFunctionType.Tanh,
                    bias=bias_t[:, blk:blk + 1],
                )
                nc.sync.dma_start(out_d[blk], outp[:])
    nc.compile()
    return nc


def _prepare(x, kernel_vector, bias, nonzero_ind):
    """Host-side shard prep. Returns (kp, per-core input dicts)."""
    x = np.asarray(x, dtype=np.float32)
    v = np.asarray(kernel_vector, dtype=np.float32).ravel()
    bias = np.asarray(bias, dtype=np.float32).ravel()
    ind = np.asarray(nonzero_ind)
    r = ind[:, 0].astype(np.int64)
    c = ind[:, 1].astype(np.int64)

    # COO .set semantics: de-duplicate (row, col), keeping the last occurrence.
    flat = r * UNITS + c
    if len(np.unique(flat)) != len(flat):
        _, last_rev = np.unique(flat[::-1], return_index=True)
        keep = np.sort(len(flat) - 1 - last_rev)
        r, c, v = r[keep], c[keep], v[keep]

    xt16 = np.ascontiguousarray(x.T).astype(np.float16)  # [INPUT_DIM, BATCH]

    # Sort by column, assign each entry its slot k within its column.
    order = np.argsort(c, kind="stable")
    r_s, c_s, v_s = r[order], c[order], v[order]
    counts = np.bincount(c_s, minlength=UNITS)
    kp = max(K_CHUNK, int(-(-counts.max() // K_CHUNK)) * K_CHUNK)
    nch = kp // K_CHUNK
    starts = np.zeros(UNITS + 1, dtype=np.int64)
    np.cumsum(counts, out=starts[1:])
    k_s = np.arange(len(c_s), dtype=np.int64) - starts[c_s]

    # Padded-CSC payload, entry-innermost per chunk: g_all[c, chunk, b, k]
    # holds the x column-vectors the entries touch (fp16); values fp16;
    # products/accumulation are f32 (GPSIMD path) / fp16-product with f32
    # accumulation (DVE path). Padding slots stay 0.
    val_all = np.zeros((UNITS, kp), dtype=np.float16)
    val_all[c_s, k_s] = v_s.astype(np.float16)
    g_all = np.zeros((UNITS, nch, BATCH, K_CHUNK), dtype=np.float16)
    g_all[c_s, k_s // K_CHUNK, :, k_s % K_CHUNK] = xt16[r_s]

    g_all = g_all.reshape(N_CORES, BLOCKS_PER_CORE, 128, nch, BATCH, K_CHUNK)
    val_all = val_all.reshape(N_CORES, BLOCKS_PER_CORE, 128, kp)
    bias2 = np.ascontiguousarray(
        bias.reshape(N_CORES, BLOCKS_PER_CORE, 128).transpose(0, 2, 1))

    in_maps = []
    for d in range(N_CORES):
        in_maps.append({
            "gvals": g_all[d],
            "vals": val_all[d],
            "bias2": bias2[d],
        })
    return kp, in_maps


def _run(inputs, trace=False):
    from concourse.bass_utils import run_bass_kernel_spmd

    kp, in_maps = _prepare(**inputs)
    if kp not in _PROGRAM_CACHE:
        _PROGRAM_CACHE[kp] = _build_program(kp)
    nc = _PROGRAM_CACHE[kp]
    res = None
    for attempt in range(3):
        try:
            res = run_bass_kernel_spmd(
                nc, in_maps, list(range(N_CORES)), trace=trace,
            )
            break
        except Exception:
            # Transient device faults (e.g. NRT_EXEC_UNIT_UNRECOVERABLE)
            # clear on re-execution; re-raise only if persistent.
            if attempt == 2:
                raise
    assert res is not None
    out_t = np.concatenate([res.results[d]["out"].reshape(UNITS_PER_CORE, BATCH)
                            for d in range(N_CORES)], axis=0)  # [2048, 32]
    out = np.ascontiguousarray(out_t.T).astype(np.float32)  # [32, 2048]
    return out, res


def kernel(**inputs):
    out, _ = _run(inputs, trace=False)
    return out



# revision 2
# speedup vs baseline: 1.3109x; 1.3109x over previous
"""Sparse-weight matmul (BiologicalModule) on 8 Trainium2 NeuronCores.

Computes: out = tanh(x @ scatter_coo(kernel_vector, nonzero_ind) + bias)
  x [32, 30000] f32, 500K COO nonzeros into a [30000, 2048] weight matrix.

Strategy (units-sharded, 256 output columns per core):
  - Never materialize the dense [30000, 2048] weight matrix. In CSC view,
    out[b, c] = sum_k v[c,k] * x[r[c,k], b].
  - Host packs a padded-CSC payload with the entry-slot axis k on SBUF
    PARTITIONS: per core, g[cs, kc, k_p, b, c] holds the x values each
    entry touches (fp16), v[k_p, kc, c] the entry values. The bias is
    folded in as one extra entry slot per column (g=1, v=bias[c]).
  - Device pipeline per column-slice cs:
      DVE : prod[k_p, (b,c)] = g * v  (v broadcast over b via a 0-stride
            middle dim; last dim step-1 fp16 keeps the 2x_1P perf mode)
      PE  : reduce over the partition (k) axis with ones-matmuls
            accumulated across the k-chunks in PSUM
            (out_mtile[128,1] = prod_tile[128k,128m]^T @ ones[128k,1])
      ACT : tanh(PSUM) -> SBUF f32, then DMA out.
    DMA-in streams ~4.2 MB/core; all compute overlaps the stream.
"""

import sys

import numpy as np

_TRN_REPO = "/opt/trn_rl_repo"
if _TRN_REPO not in sys.path:
    sys.path.insert(0, _TRN_REPO)

INPUT_DIM = 30000
UNITS = 2048
BATCH = 32
N_CORES = 8
UPC = UNITS // N_CORES  # 256 columns per core
CS = 32  # columns per slice
NCS = UPC // CS  # 8 slices per core
MT = BATCH * CS // 128  # 8 m-tiles (PE matmuls) per slice per k-chunk
G_BUFS = 6
PROD_BUFS = 4

_PROGRAM_CACHE = {}


def _build_program(nkc):
    """Build + compile the SPMD bass program for nkc 128-slot k-chunks."""
    from concourse import bacc, tile
    from concourse.bass import AP
    import concourse.mybir as mybir

    f32 = mybir.dt.float32
    f16 = mybir.dt.float16
    fcs = BATCH * CS  # free size of one (cs, kc) tile

    nc = bacc.Bacc("TRN2", target_bir_lowering=False, debug=False,
                   num_devices=N_CORES)
    g_d = nc.dram_tensor("gvals", [NCS, nkc, 128, fcs], f16,
                         kind="ExternalInput")
    v_d = nc.dram_tensor("vals", [128, nkc * UPC], f16, kind="ExternalInput")
    out_d = nc.dram_tensor("out", [NCS, 128, MT], f32, kind="ExternalOutput")

    with tile.TileContext(nc) as tc:
        with (
            tc.tile_pool(name="persist", bufs=1) as persist,
            tc.tile_pool(name="gwork", bufs=G_BUFS) as gwork,
            tc.tile_pool(name="pwork", bufs=PROD_BUFS) as pwork,
            tc.tile_pool(name="owork", bufs=2) as owork,
            tc.psum_pool(name="psum", bufs=2) as psum,
        ):
            v_t = persist.tile([128, nkc * UPC], f16, tag="v")
            nc.sync.dma_start(v_t[:], v_d[:])
            ones = persist.tile([128, 1], f16, tag="ones")
            nc.vector.memset(ones[:], 1.0)
            for cs in range(NCS):
                prods = []
                for kc in range(nkc):
                    g_t = gwork.tile([128, fcs], f16, tag="g",
                                     name=f"g{cs}_{kc}")
                    nc.sync.dma_start(g_t[:], g_d[cs, kc])
                    prod = pwork.tile([128, fcs], f16, tag="prod",
                                      name=f"prod{cs}_{kc}")
                    # v operand viewed [k_p, b(step 0), c(step 1)]
                    base = v_t[:, kc * UPC + cs * CS: kc * UPC + (cs + 1) * CS]
                    v_bc = AP(base.tensor, base.offset,
                              [base.ap[0], [0, BATCH], base.ap[1]])
                    nc.vector.tensor_tensor(prod[:], g_t[:], v_bc,
                                            mybir.AluOpType.mult)
                    prods.append(prod)
                ps = psum.tile([128, MT], f32, tag="ps", name=f"ps{cs}")
                for j in range(MT):
                    for kc in range(nkc):
                        nc.tensor.matmul(
                            ps[:, j:j + 1],
                            lhsT=prods[kc][:, 128 * j:128 * (j + 1)],
                            rhs=ones[:, 0:1],
                            start=(kc == 0),
                            stop=(kc == nkc - 1),
                        )
                o_t = owork.tile([128, MT], f32, tag="o", name=f"o{cs}")
                nc.scalar.activation(o_t[:], ps[:],
                                     mybir.ActivationFunctionType.Tanh)
                nc.scalar.dma_start(out_d[cs], o_t[:])
    nc.compile()
    return nc


def _prepare(x, kernel_vector, bias, nonzero_ind):
    """Host-side shard prep. Returns (nkc, per-core input dicts)."""
    x = np.asarray(x, dtype=np.float32)
    v = np.asarray(kernel_vector, dtype=np.float32).ravel()
    bias = np.asarray(bias, dtype=np.float32).ravel()
    ind = np.asarray(nonzero_ind)
    r = ind[:, 0].astype(np.int64)
    c = ind[:, 1].astype(np.int64)

    # COO .set semantics: de-duplicate (row, col), keeping the last occurrence.
    flat = r * UNITS + c
    if len(np.unique(flat)) != len(flat):
        _, last_rev = np.unique(flat[::-1], return_index=True)
        keep = np.sort(len(flat) - 1 - last_rev)
        r, c, v = r[keep], c[keep], v[keep]

    xt16 = np.ascontiguousarray(x.T).astype(np.float16)  # [INPUT_DIM, BATCH]

    # Sort by column, assign each entry its slot k within its column.
    order = np.argsort(c, kind="stable")
    r_s, c_s, v_s = r[order], c[order], v[order]
    counts = np.bincount(c_s, minlength=UNITS)
    # +1 slot per column for the bias entry
    nkc = max(1, -(-(int(counts.max()) + 1) // 128))
    kp = nkc * 128
    starts = np.zeros(UNITS + 1, dtype=np.int64)
    np.cumsum(counts, out=starts[1:])
    k_s = np.arange(len(c_s), dtype=np.int64) - starts[c_s]

    # g_full[c, k, b]: x row for the entry at (column c, slot k); padding 0.
    g_full = np.zeros((UNITS, kp, BATCH), dtype=np.float16)
    g_full[c_s, k_s] = xt16[r_s]
    v_full = np.zeros((UNITS, kp), dtype=np.float16)
    v_full[c_s, k_s] = v_s.astype(np.float16)
    # bias as one extra entry: value bias[c], "x vector" of ones
    cols = np.arange(UNITS)
    g_full[cols, counts] = np.float16(1.0)
    v_full[cols, counts] = bias.astype(np.float16)

    # -> per-core [NCS, nkc, 128(k_in), BATCH*CS] with free dim (b, c_local)
    g_all = g_full.reshape(N_CORES, NCS, CS, nkc, 128, BATCH)
    g_all = np.ascontiguousarray(g_all.transpose(0, 1, 3, 4, 5, 2))
    g_all = g_all.reshape(N_CORES, NCS, nkc, 128, BATCH * CS)
    # v -> per-core [128(k_in), nkc*UPC(c)]
    v_all = v_full.reshape(N_CORES, UPC, nkc, 128)
    v_all = np.ascontiguousarray(v_all.transpose(0, 3, 2, 1))
    v_all = v_all.reshape(N_CORES, 128, nkc * UPC)

    in_maps = []
    for d in range(N_CORES):
        in_maps.append({"gvals": g_all[d], "vals": v_all[d]})
    return nkc, in_maps


def _unshard(res):
    """[d][NCS, 128, MT] f32 -> [BATCH, UNITS] f32.

    Within m-tile j, partition p holds flat index f = 128*j + p of the
    (b-major, c_local-minor) flattening of [BATCH, CS].
    """
    out_arr = np.stack([np.asarray(res.results[d]["out"]).reshape(NCS, 128, MT)
                        for d in range(N_CORES)], axis=0)  # [d, cs, p, j]
    out_arr = out_arr.transpose(0, 1, 3, 2).reshape(N_CORES, NCS, MT * 128)
    # flat f = b*CS + c_local
    out_arr = out_arr.reshape(N_CORES, NCS, BATCH, CS)
    out = out_arr.transpose(2, 0, 1, 3).reshape(BATCH, UNITS)
    return np.ascontiguousarray(out).astype(np.float32)


def _run(inputs, trace=False):
    from concourse.bass_utils import run_bass_kernel_spmd

    nkc, in_maps = _prepare(**inputs)
    if nkc not in _PROGRAM_CACHE:
        _PROGRAM_CACHE[nkc] = _build_program(nkc)
    nc = _PROGRAM_CACHE[nkc]
    res = None
    for attempt in range(3):
        try:
            res = run_bass_kernel_spmd(
                nc, in_maps, list(range(N_CORES)), trace=trace,
            )
            break
        except Exception:
            # Transient device faults (e.g. NRT_EXEC_UNIT_UNRECOVERABLE)
            # clear on re-execution; re-raise only if persistent.
            if attempt == 2:
                raise
    assert res is not None
    return _unshard(res), res


def kernel(**inputs):
    out, _ = _run(inputs, trace=False)
    return out
